# revision 11
# baseline (speedup 1.0000x reference)
"""GCN layer on 8 Trainium2 NeuronCores (Bass/Tile, SPMD).

  H = X @ W^T + b                      (dense projection, fp16)
  out[r] = sum_{e: row[e]==r} val[e] * H[col[e]]   (gather + segment-sum)

Sharding: destination nodes (rows of output) are sharded across the 8
cores (12500 rows each); W/b replicated; each core computes the full
projection H (replicated) into its own HBM, then gathers source rows for
its edge shard with SWDGE dma_gather and reduces them per 128-row
destination group with a selector-matrix matmul accumulated in PSUM.

Device-side layout tricks:
  * H is stored interleaved in HBM as H_hbm[p, t, f] = H[t*128+p, f]
    (shape [128, T_T, 128] fp16).  Projection PSUM tiles map directly to
    H_hbm[:, t, :] so H writes are large line-rate DMAs, and the gather
    row index becomes the host-side permutation (col%128)*T_T + col//128.
  * dma_gather indices are int16, so the gather is chunked 4 ways by
    col%128 in [32c, 32c+32) -> local index < 32*T_T = 25024 < 32768.
  * Edges are bucketed by (dest-group g, chunk c) and each bucket padded
    to a multiple of 128 with val=0 edges; the per-(g,c) tile count is
    the max across all 8 cores so a single SPMD program serves them all.
  * A 128-edge tile is reduced into its destination group's PSUM tile by
    matmul(lhsT=S_T, rhs=M) where S_T[e, r] = (lr[e] == r) is built on
    DVE via one is_equal over a broadcast iota; edge values are folded
    into the gathered messages with one tensor_tensor multiply per block.
"""

import numpy as np

from concourse import bacc, mybir, tile
from concourse.bass_utils import run_bass_kernel_spmd

dt = mybir.dt

# ---------------------------------------------------------------- constants
N_NODES = 100000
IN_DIM = 256
OUT_DIM = 128
N_EDGES = 1600000
N_CORES = 8
P = 128


class Cfg:
    def __init__(self, n_nodes=N_NODES, rows_per_core=12500, block_groups=8,
                 max_gather_slots=8, n_queues=4, dma_scratch=16384):
        self.n_nodes = n_nodes
        self.rows_per_core = rows_per_core
        self.max_gather_slots = max_gather_slots
        self.n_queues = n_queues
        self.dma_scratch = dma_scratch
        self.t_t = -(-n_nodes // P)              # projection row tiles
        self.n_nodes_pad = self.t_t * P
        self.ng = -(-rows_per_core // P)         # dest groups per core
        self.rows_pad = self.ng * P
        self.block_groups = block_groups
        self.chunk_rows = 32 * self.t_t          # gather rows per chunk
        assert self.chunk_rows < 32768, "int16 gather index overflow"


DEFAULT_CFG = Cfg()


# ---------------------------------------------------------------- host side
def build_schedule(cfg, edge_row, edge_col, edge_val):
    """Bucket edges by (core, dest-group, chunk); build the shared SPMD
    schedule (cross-core max tile counts) and per-core slot arrays."""
    er = np.asarray(edge_row).astype(np.int64)
    ec = np.asarray(edge_col).astype(np.int64)
    ev = np.asarray(edge_val).astype(np.float32)

    core = er // cfg.rows_per_core
    lr = er - core * cfg.rows_per_core
    g = lr // P
    lrg = (lr - g * P).astype(np.float16)        # in-group row, exact in fp16
    p_src = ec % P
    t_src = ec // P
    c = p_src // 32
    idx16 = ((p_src - 32 * c) * cfg.t_t + t_src).astype(np.int16)
    val16 = ev.astype(np.float16)

    ng, nch = cfg.ng, 4
    bucket = ((core * ng + g) * nch + c).astype(np.int64)
    order = np.argsort(bucket, kind="stable")
    cnt = np.bincount(bucket, minlength=N_CORES * ng * nch).reshape(N_CORES, ng, nch)
    starts = np.zeros((N_CORES, ng, nch), dtype=np.int64)
    flat = cnt.reshape(-1)
    np.cumsum(flat[:-1], out=starts.reshape(-1)[1:])

    idx16_s = idx16[order]
    lrg_s = lrg[order]
    val16_s = val16[order]

    T = np.ceil(cnt / P).astype(np.int64).max(axis=0)      # [ng, nch]

    # block / section structure (shared by all cores)
    blocks = [list(range(b, min(b + cfg.block_groups, ng)))
              for b in range(0, ng, cfg.block_groups)]
    sections = []          # (blk_id, c, sec_slot_start_within_block, n_slots, col8_off)
    tot_slots = int(T.sum())
    tot8 = tot_slots * 8
    # group-major LRG offsets
    g_tiles = T.sum(axis=1)                                 # tiles per group
    g_qoff = np.zeros(ng, dtype=np.int64)
    np.cumsum(g_tiles[:-1], out=g_qoff[1:])

    slot_cursor = 0
    col8_cursor = 0
    blk_meta = []
    for bi, blk in enumerate(blocks):
        secs = []
        blk_slot0 = slot_cursor
        for cc in range(nch):
            n_slots = int(T[blk, cc].sum())
            secs.append((cc, slot_cursor - blk_slot0, n_slots, col8_cursor,
                         slot_cursor))
            slot_cursor += n_slots
            col8_cursor += n_slots * 8
        # per-group tile -> global slot map
        gmap = {}
        for gi in blk:
            slots = []
            for cc in range(nch):
                sec_off = secs[cc][4]
                o_gc = int(T[[x for x in blk if x < gi], cc].sum())
                for k in range(int(T[gi, cc])):
                    slots.append(sec_off + o_gc + k)
            gmap[gi] = slots
        blk_meta.append(dict(blk=blk, secs=secs, blk_slot0=blk_slot0,
                             n_slots=slot_cursor - blk_slot0, gmap=gmap))
    assert slot_cursor == tot_slots

    # per-core data arrays
    cols = np.zeros((N_CORES, 16, tot8), dtype=np.int16)
    # cols layout: per section, j-th idx -> [j%16, col8_off + j//16]
    lrg_arr = np.zeros((N_CORES, P, tot_slots), dtype=np.float16)   # group-major
    val_arr = np.zeros((N_CORES, P, tot_slots), dtype=np.float16)   # slot-major

    for ci in range(N_CORES):
        for bm in blk_meta:
            for (cc, _soff, n_slots, col8_off, sec_slot0) in bm["secs"]:
                if n_slots == 0:
                    continue
                # assemble the section's padded edge stream
                parts_idx = []
                parts_val = []
                parts_lrg = []
                for gi in bm["blk"]:
                    n = int(cnt[ci, gi, cc])
                    s0 = int(starts[ci, gi, cc])
                    tgt = int(T[gi, cc]) * P
                    bi_ = idx16_s[s0:s0 + n]
                    bv = val16_s[s0:s0 + n]
                    bl = lrg_s[s0:s0 + n]
                    if tgt > n:
                        pad_idx = bi_[-1] if n > 0 else np.int16(cc * 0)
                        bi_ = np.concatenate([bi_, np.full(tgt - n, pad_idx, np.int16)])
                        bv = np.concatenate([bv, np.zeros(tgt - n, np.float16)])
                        bl = np.concatenate([bl, np.zeros(tgt - n, np.float16)])
                    parts_idx.append(bi_)
                    parts_val.append(bv)
                    parts_lrg.append(bl)
                sidx = np.concatenate(parts_idx)
                sval = np.concatenate(parts_val)
                slrg = np.concatenate(parts_lrg)
                n_e = n_slots * P
                assert sidx.shape[0] == n_e
                # cols: 16-partition wrap
                cols[ci, :, col8_off:col8_off + n_slots * 8] = \
                    sidx.reshape(n_e // 16, 16).T
                # val: M layout (partition j%128, slot sec_slot0 + j//128)
                val_arr[ci, :, sec_slot0:sec_slot0 + n_slots] = \
                    sval.reshape(n_slots, P).T
                # lrg: group-major layout
                q = 0
                for gi in bm["blk"]:
                    tg = int(T[gi, cc]) * P
                    qcol = g_qoff[gi] + int(T[gi, :cc].sum())
                    lrg_arr[ci, :, qcol:qcol + tg // P] = \
                        slrg[q:q + tg].reshape(tg // P, P).T
                    q += tg

    cols_full = np.tile(cols, (1, 8, 1))     # replicate to 128 partitions

    sched = dict(T=T, blocks=blocks, blk_meta=blk_meta, tot_slots=tot_slots,
                 tot8=tot8, g_qoff=g_qoff, g_tiles=g_tiles,
                 slots_blk_max=max(bm["n_slots"] for bm in blk_meta))
    data = dict(cols=cols_full, lrg=lrg_arr, val=val_arr)
    return sched, data


# ---------------------------------------------------------------- device side
def build_program(cfg, sched):
    nc = bacc.Bacc("TRN2", target_bir_lowering=False, debug=False,
                   num_swdge_queues=cfg.n_queues,
                   dynamic_dma_scratch_size=cfg.dma_scratch)
    f16, f32 = dt.float16, dt.float32

    XT = nc.dram_tensor("xt", [IN_DIM, cfg.n_nodes_pad], f16, kind="ExternalInput")
    WT = nc.dram_tensor("wt", [IN_DIM, OUT_DIM], f16, kind="ExternalInput")
    BR = nc.dram_tensor("brow", [1, OUT_DIM], f16, kind="ExternalInput")
    ON = nc.dram_tensor("ones", [1, P], f16, kind="ExternalInput")
    IO = nc.dram_tensor("iota", [P, P], f16, kind="ExternalInput")
    CO = nc.dram_tensor("cols", [P, sched["tot8"]], dt.int16, kind="ExternalInput")
    LG = nc.dram_tensor("lrg", [P, sched["tot_slots"]], f16, kind="ExternalInput")
    VA = nc.dram_tensor("val", [P, sched["tot_slots"]], f16, kind="ExternalInput")
    OUT = nc.dram_tensor("out", [cfg.rows_pad, OUT_DIM], f32, kind="ExternalOutput")

    T = sched["T"]
    blk_meta = sched["blk_meta"]
    t_t = cfg.t_t

    with tile.TileContext(nc) as tc:
        with (
            tc.tile_pool(name="dram", bufs=1, space="DRAM") as dpool,
            tc.tile_pool(name="const", bufs=1) as cpool,
        ):
            H = dpool.tile([P, t_t, OUT_DIM], f16)
            H_flat = H[:].rearrange("p t f -> (p t) f")

            wt0 = cpool.tile([P, OUT_DIM], f16)
            wt1 = cpool.tile([P, OUT_DIM], f16)
            nc.sync.dma_start(out=wt0[:], in_=WT[0:P, :])
            nc.sync.dma_start(out=wt1[:], in_=WT[P:2 * P, :])
            ones_t = cpool.tile([1, P], f16)
            brow_t = cpool.tile([1, OUT_DIM], f16)
            nc.sync.dma_start(out=ones_t[:], in_=ON[:, :])
            nc.sync.dma_start(out=brow_t[:], in_=BR[:, :])
            iota_t = cpool.tile([P, P], f16)
            nc.sync.dma_start(out=iota_t[:], in_=IO[:, :])
            cols_t = cpool.tile([P, sched["tot8"]], dt.int16)
            nc.sync.dma_start(out=cols_t[:], in_=CO[:, :])
            lrg_t = cpool.tile([P, sched["tot_slots"]], f16)
            nc.sync.dma_start(out=lrg_t[:], in_=LG[:, :])
            val_t = cpool.tile([P, sched["tot_slots"]], f16)
            nc.sync.dma_start(out=val_t[:], in_=VA[:, :])

            # ---------------- phase 1: H = X @ W^T + b (interleaved layout)
            QUAD = 4              # row tiles per PSUM bank / H write
            SLAB = 96             # row tiles per XT slab DMA
            with (
                tc.tile_pool(name="xt", bufs=2) as xtp,
                tc.tile_pool(name="hpsum", bufs=4, space="PSUM") as hpp,
                tc.tile_pool(name="hstg", bufs=3) as hsp,
            ):
                for s0 in range(0, t_t, SLAB):
                    s1 = min(s0 + SLAB, t_t)
                    rows = (s1 - s0) * P
                    xt0 = xtp.tile([P, SLAB * P], f16, tag="x0")
                    xt1 = xtp.tile([P, SLAB * P], f16, tag="x1")
                    nc.sync.dma_start(out=xt0[:, :rows], in_=XT[0:P, s0 * P:s1 * P])
                    nc.sync.dma_start(out=xt1[:, :rows], in_=XT[P:2 * P, s0 * P:s1 * P])
                    for q0 in range(s0, s1, QUAD):
                        q1 = min(q0 + QUAD, s1)
                        nq = q1 - q0
                        ps = hpp.tile([P, QUAD * OUT_DIM], f32)
                        for qi, t in enumerate(range(q0, q1)):
                            o = (t - s0) * P
                            seg = ps[:, qi * OUT_DIM:(qi + 1) * OUT_DIM]
                            nc.tensor.matmul(seg, lhsT=xt0[:, o:o + P], rhs=wt0[:],
                                             start=True, stop=False)
                            nc.tensor.matmul(seg, lhsT=xt1[:, o:o + P], rhs=wt1[:],
                                             start=False, stop=False)
                            nc.tensor.matmul(seg, lhsT=ones_t[:1, :P], rhs=brow_t[:1, :],
                                             start=False, stop=True)
                        hst = hsp.tile([P, QUAD, OUT_DIM], f16)
                        nc.any.tensor_copy(
                            hst[:, :nq, :],
                            ps[:, :nq * OUT_DIM].rearrange("p (q f) -> p q f", f=OUT_DIM))
                        nc.sync.dma_start(out=H[:, q0:q1, :], in_=hst[:, :nq, :])

            # ---------------- phase 2: gather + selector-matmul segment sum
            smax = sched["slots_blk_max"]
            with (
                tc.tile_pool(name="msgs", bufs=2) as mpool,
                tc.tile_pool(name="spsum", bufs=4, space="PSUM") as spp,
                tc.tile_pool(name="st", bufs=3) as stp,
                tc.tile_pool(name="ostg", bufs=3) as opool,
            ):
                gq = 0
                for bm in blk_meta:
                    nsl = bm["n_slots"]
                    if nsl == 0:
                        continue
                    blk_slot0 = bm["blk_slot0"]
                    mt = mpool.tile([P, smax, OUT_DIM], f16, tag="m")
                    for (cc, soff, n_slots, col8_off, _sec0) in bm["secs"]:
                        for a in range(0, n_slots, cfg.max_gather_slots):
                            k = min(cfg.max_gather_slots, n_slots - a)
                            nc.gpsimd.dma_gather(
                                out_ap=mt[:, soff + a:soff + a + k, :],
                                in_ap=H_flat[cc * cfg.chunk_rows:(cc + 1) * cfg.chunk_rows, :],
                                idxs_ap=cols_t[:, col8_off + a * 8:col8_off + (a + k) * 8],
                                num_idxs=k * P,
                                num_idxs_reg=k * P,
                                elem_size=OUT_DIM,
                                queue_num=gq % cfg.n_queues,
                            )
                            gq += 1
                    # fold edge values into the messages
                    nc.any.tensor_tensor(
                        out=mt[:, :nsl, :],
                        in0=mt[:, :nsl, :],
                        in1=val_t[:, blk_slot0:blk_slot0 + nsl, None]
                        .to_broadcast([P, nsl, OUT_DIM]),
                        op=mybir.AluOpType.mult,
                    )
                    for gi in bm["blk"]:
                        slots = bm["gmap"][gi]
                        tg = len(slots)
                        og = opool.tile([P, OUT_DIM], f32)
                        if tg == 0:
                            nc.any.memset(og[:], 0.0)
                        else:
                            q0 = int(sched["g_qoff"][gi])
                            st = stp.tile([P, max(tg, 1) * P], f16, tag="st")
                            nc.vector.tensor_tensor(
                                out=st[:, :tg * P].rearrange("p (t r) -> p t r", r=P),
                                in0=lrg_t[:, q0:q0 + tg, None].to_broadcast([P, tg, P]),
                                in1=iota_t[:, None, :].to_broadcast([P, tg, P]),
                                op=mybir.AluOpType.is_equal,
                            )
                            ps = spp.tile([P, OUT_DIM], f32)
                            for k, slot in enumerate(slots):
                                nc.tensor.matmul(
                                    ps[:],
                                    lhsT=st[:, k * P:(k + 1) * P],
                                    rhs=mt[:, slot - blk_slot0, :],
                                    start=(k == 0),
                                    stop=(k == tg - 1),
                                )
                            nc.any.tensor_copy(og[:], ps[:])
                        nc.sync.dma_start(out=OUT[gi * P:(gi + 1) * P, :], in_=og[:])

    nc.finalize()
    return nc


# ---------------------------------------------------------------- driver
_CACHE = {}


def _ensure_ntff_hook():
    """Provide antenv.axon_hooks + the ctypes NTFF profile hook when the
    agent image lacks them (needed only for trace=True)."""
    import sys
    import types
    import contextlib
    import ctypes
    try:
        from antenv.axon_hooks import get_axon_ntff_profile_hook  # noqa: F401
        return
    except ImportError:
        pass
    import antenv
    mod = types.ModuleType("antenv.axon_hooks")
    mod._hook = None

    def set_axon_ntff_profile_hook(h):
        mod._hook = h

    def get_axon_ntff_profile_hook():
        return mod._hook

    mod.set_axon_ntff_profile_hook = set_axon_ntff_profile_hook
    mod.get_axon_ntff_profile_hook = get_axon_ntff_profile_hook
    sys.modules["antenv.axon_hooks"] = mod
    antenv.axon_hooks = mod

    so_path = "/opt/axon/libaxon_pjrt.so"
    try:
        lib = ctypes.CDLL(so_path)
    except OSError:
        return
    if not hasattr(lib, "axon_start_nrt_profile"):
        return
    lib.axon_start_nrt_profile.argtypes = [ctypes.POINTER(ctypes.c_int64),
                                           ctypes.c_size_t]
    lib.axon_start_nrt_profile.restype = ctypes.c_int64
    lib.axon_stop_nrt_profile.argtypes = [ctypes.c_char_p]
    lib.axon_stop_nrt_profile.restype = ctypes.c_int64

    @contextlib.contextmanager
    def _hook(output_dir, device_ids):
        import jax
        jax.devices()
        if device_ids:
            ids = (ctypes.c_int64 * len(device_ids))(*device_ids)
            rc = lib.axon_start_nrt_profile(ids, len(device_ids))
        else:
            rc = lib.axon_start_nrt_profile(None, 0)
        if rc != 0:
            raise RuntimeError(f"axon_start_nrt_profile rc={rc}")
        try:
            yield
        finally:
            n = lib.axon_stop_nrt_profile(str(output_dir).encode())
            print(f"ntff profile: {n} file(s) written to {output_dir}", flush=True)

    set_axon_ntff_profile_hook(_hook)


def _prep_inputs(cfg, X, W, b, sched, data):
    xt = np.zeros((IN_DIM, cfg.n_nodes_pad), dtype=np.float16)
    xt[:, :cfg.n_nodes] = np.asarray(X, np.float32).T.astype(np.float16)
    wt = np.asarray(W, np.float32).T.astype(np.float16)          # [256,128]
    brow = np.asarray(b, np.float32).astype(np.float16)[None, :]
    ones = np.ones((1, P), dtype=np.float16)
    iota = np.tile(np.arange(P, dtype=np.float16)[None, :], (P, 1))
    in_maps = []
    for ci in range(N_CORES):
        in_maps.append({
            "xt": xt, "wt": wt, "brow": brow, "ones": ones, "iota": iota,
            "cols": data["cols"][ci], "lrg": data["lrg"][ci],
            "val": data["val"][ci],
        })
    return in_maps


def run(X, edge_row, edge_col, edge_val, W, b, cfg=DEFAULT_CFG, trace=False):
    if trace:
        _ensure_ntff_hook()
    sched, data = build_schedule(cfg, edge_row, edge_col, edge_val)
    key = ("prog", cfg.n_nodes, cfg.rows_per_core, sched["tot_slots"],
           tuple(sched["T"].reshape(-1)))
    if key not in _CACHE:
        _CACHE.clear()
        _CACHE[key] = build_program(cfg, sched)
    nc = _CACHE[key]
    in_maps = _prep_inputs(cfg, X, W, b, sched, data)
    res = run_bass_kernel_spmd(nc, in_maps, core_ids=list(range(N_CORES)),
                               trace=trace)
    outs = [res.results[ci]["out"][:cfg.rows_per_core] for ci in range(N_CORES)]
    full = np.concatenate(outs, axis=0).astype(np.float32)
    return full, res


def kernel(X, edge_row, edge_col, edge_val, W, b):
    out, _ = run(X, edge_row, edge_col, edge_val, W, b)
    return out


# revision 21
# speedup vs baseline: 1.3592x; 1.3592x over previous
"""GCN layer on 8 Trainium2 NeuronCores (Bass/Tile, SPMD).

  H' = X @ W^T                                  (dense projection, fp16)
  out[r] = sum_{e: row[e]==r} val[e] * H'[col[e]]  +  deg[r] * b
  where deg[r] = sum_{e: row[e]==r} val[e]     (bias folded via degree)

Sharding: destination nodes (rows of the output) are sharded across the
8 cores (12500 rows each); W/b replicated; each core computes the full
projection H' (replicated) into its own HBM, then gathers source rows
for its edge shard with SWDGE dma_gather and reduces them per 128-row
destination group with a selector-matrix matmul accumulated in PSUM.

Device-side layout:
  * Nodes are assigned to 4 chunks by n % 4 and renumbered q = n // 4.
    Chunk c's projection occupies a contiguous range of row-tiles, is
    written to its own DRAM buffer H_c[p, u, f] = H'[node(c, u*128+p)]
    (so PSUM tiles map 1:1 to large line-rate writes), and unblocks that
    chunk's gathers while later chunks are still projecting.
  * dma_gather indices are int16: the in-chunk index
    (q%128)*T_C + q//128 < 128*T_C = 25088 always fits.
  * Edges are bucketed by (dest-group g, chunk c); each bucket is padded
    to a multiple of 128 with val=0 copies of its last edge; the tile
    count per bucket is the max across all 8 cores so one SPMD program
    serves every core.
  * A 128-edge tile is reduced into its destination group's PSUM via
    matmul(lhsT=S_T, rhs=M) with S_T[e, r] = (lr[e] == r) built on DVE
    by one is_equal against a broadcast iota per group; edge values are
    folded into the gathered messages with one multiply per section.
  * Each group's PSUM accumulation starts with a rank-1 matmul
    deg_g (x) b that contributes the bias term.
"""

import numpy as np

from concourse import bacc, mybir, tile
from concourse.bass_utils import run_bass_kernel_spmd

dt = mybir.dt

# ---------------------------------------------------------------- constants
N_NODES = 100000
IN_DIM = 256
OUT_DIM = 128
N_EDGES = 1600000
N_CORES = 8
P = 128
NCH = 4


class Cfg:
    def __init__(self, n_nodes=N_NODES, rows_per_core=12500, block_groups=8,
                 max_gather_slots=8, n_queues=4, dma_scratch=16384):
        self.n_nodes = n_nodes
        self.rows_per_core = rows_per_core
        self.max_gather_slots = max_gather_slots
        self.n_queues = n_queues
        self.dma_scratch = dma_scratch
        self.chunk_nodes = -(-n_nodes // NCH)        # nodes per chunk (n % 4)
        self.t_c = -(-self.chunk_nodes // P)         # row tiles per chunk
        self.chunk_rows_pad = self.t_c * P
        assert self.chunk_rows_pad < 32768, "int16 gather index overflow"
        self.t_t = NCH * self.t_c                    # total projection tiles
        self.n_nodes_pad = self.t_t * P
        self.ng = -(-rows_per_core // P)             # dest groups per core
        self.rows_pad = self.ng * P
        self.block_groups = block_groups


DEFAULT_CFG = Cfg()


# ---------------------------------------------------------------- host side
def build_schedule(cfg, edge_row, edge_col, edge_val):
    """Bucket edges by (core, dest-group, chunk); build the shared SPMD
    schedule (cross-core max tile counts) and per-core data arrays."""
    er = np.asarray(edge_row).astype(np.int64)
    ec = np.asarray(edge_col).astype(np.int64)
    ev = np.asarray(edge_val).astype(np.float32)

    core = er // cfg.rows_per_core
    lr = er - core * cfg.rows_per_core
    g = lr // P
    lrg = (lr - g * P).astype(np.float16)        # in-group row, exact in fp16
    c = ec % NCH
    q = ec // NCH
    idx16 = ((q % P) * cfg.t_c + q // P).astype(np.int16)
    val16 = ev.astype(np.float16)

    ng = cfg.ng
    bucket = ((core * ng + g) * NCH + c).astype(np.int64)
    order = np.argsort(bucket, kind="stable")
    cnt = np.bincount(bucket, minlength=N_CORES * ng * NCH).reshape(N_CORES, ng, NCH)
    starts = np.zeros((N_CORES, ng, NCH), dtype=np.int64)
    np.cumsum(cnt.reshape(-1)[:-1], out=starts.reshape(-1)[1:])

    idx16_s = idx16[order]
    lrg_s = lrg[order]
    val16_s = val16[order]

    T = np.ceil(cnt / P).astype(np.int64).max(axis=0)      # [ng, NCH]

    blocks = [list(range(b, min(b + cfg.block_groups, ng)))
              for b in range(0, ng, cfg.block_groups)]
    tot_slots = int(T.sum())
    tot8 = tot_slots * 8
    g_tiles = T.sum(axis=1)
    g_qoff = np.zeros(ng, dtype=np.int64)
    np.cumsum(g_tiles[:-1], out=g_qoff[1:])

    # section = (block, chunk): contiguous slot range in the global arrays
    slot_cursor = 0
    blk_meta = []
    for blk in blocks:
        secs = []
        blk_slot0 = slot_cursor
        for cc in range(NCH):
            n_slots = int(T[blk, cc].sum())
            secs.append(dict(c=cc, n_slots=n_slots, slot0=slot_cursor,
                             col8_off=slot_cursor * 8))
            slot_cursor += n_slots
        # per-group: list of (c, section-local slot, k-index in group-major S)
        gmap = {}
        for gi in blk:
            ents = []
            kk = 0
            for cc in range(NCH):
                o_gc = int(T[[x for x in blk if x < gi], cc].sum())
                for k in range(int(T[gi, cc])):
                    ents.append((cc, o_gc + k))
                    kk += 1
            gmap[gi] = ents
        blk_meta.append(dict(blk=blk, secs=secs, blk_slot0=blk_slot0,
                             n_slots=slot_cursor - blk_slot0, gmap=gmap))
    assert slot_cursor == tot_slots
    sec_max = max((s["n_slots"] for bm in blk_meta for s in bm["secs"]),
                  default=1)

    cols = np.zeros((N_CORES, 16, tot8), dtype=np.int16)
    lrg_arr = np.zeros((N_CORES, P, tot_slots), dtype=np.float16)   # group-major
    val_arr = np.zeros((N_CORES, P, tot_slots), dtype=np.float16)   # slot-major
    deg_arr = np.zeros((N_CORES, 1, cfg.rows_pad), dtype=np.float16)

    for ci in range(N_CORES):
        m = core == ci
        deg = np.bincount(lr[m], weights=ev[m], minlength=cfg.rows_pad)
        deg_arr[ci, 0] = deg.astype(np.float16)
        for bm in blk_meta:
            for sec in bm["secs"]:
                cc, n_slots = sec["c"], sec["n_slots"]
                if n_slots == 0:
                    continue
                parts_idx, parts_val, parts_lrg = [], [], []
                for gi in bm["blk"]:
                    n = int(cnt[ci, gi, cc])
                    s0 = int(starts[ci, gi, cc])
                    tgt = int(T[gi, cc]) * P
                    bi_ = idx16_s[s0:s0 + n]
                    bv = val16_s[s0:s0 + n]
                    bl = lrg_s[s0:s0 + n]
                    if tgt > n:
                        pad_idx = bi_[-1] if n > 0 else np.int16(0)
                        bi_ = np.concatenate([bi_, np.full(tgt - n, pad_idx, np.int16)])
                        bv = np.concatenate([bv, np.zeros(tgt - n, np.float16)])
                        bl = np.concatenate([bl, np.zeros(tgt - n, np.float16)])
                    parts_idx.append(bi_)
                    parts_val.append(bv)
                    parts_lrg.append(bl)
                sidx = np.concatenate(parts_idx)
                sval = np.concatenate(parts_val)
                slrg = np.concatenate(parts_lrg)
                n_e = n_slots * P
                assert sidx.shape[0] == n_e
                cols[ci, :, sec["col8_off"]:sec["col8_off"] + n_slots * 8] = \
                    sidx.reshape(n_e // 16, 16).T
                val_arr[ci, :, sec["slot0"]:sec["slot0"] + n_slots] = \
                    sval.reshape(n_slots, P).T
                qo = 0
                for gi in bm["blk"]:
                    tg = int(T[gi, cc]) * P
                    qcol = g_qoff[gi] + int(T[gi, :cc].sum())
                    lrg_arr[ci, :, qcol:qcol + tg // P] = \
                        slrg[qo:qo + tg].reshape(tg // P, P).T
                    qo += tg

    cols_full = np.tile(cols, (1, 8, 1))

    sched = dict(T=T, blocks=blocks, blk_meta=blk_meta, tot_slots=tot_slots,
                 tot8=tot8, g_qoff=g_qoff, g_tiles=g_tiles, sec_max=sec_max,
                 tg_max=int(g_tiles.max()) if ng else 1)
    data = dict(cols=cols_full, lrg=lrg_arr, val=val_arr, deg=deg_arr)
    return sched, data


# ---------------------------------------------------------------- device side
def build_program(cfg, sched):
    nc = bacc.Bacc("TRN2", target_bir_lowering=False, debug=False,
                   num_swdge_queues=cfg.n_queues,
                   dynamic_dma_scratch_size=cfg.dma_scratch)
    f16, f32 = dt.float16, dt.float32

    XT = nc.dram_tensor("xt", [IN_DIM, cfg.n_nodes_pad], f16, kind="ExternalInput")
    WT = nc.dram_tensor("wt", [IN_DIM, OUT_DIM], f16, kind="ExternalInput")
    BR = nc.dram_tensor("brow", [1, OUT_DIM], f16, kind="ExternalInput")
    IO = nc.dram_tensor("iota", [P, P], f16, kind="ExternalInput")
    CO = nc.dram_tensor("cols", [P, sched["tot8"]], dt.int16, kind="ExternalInput")
    LG = nc.dram_tensor("lrg", [P, sched["tot_slots"]], f16, kind="ExternalInput")
    VA = nc.dram_tensor("val", [P, sched["tot_slots"]], f16, kind="ExternalInput")
    DG = nc.dram_tensor("deg", [1, cfg.rows_pad], f16, kind="ExternalInput")
    OUT = nc.dram_tensor("out", [cfg.rows_pad, OUT_DIM], f32, kind="ExternalOutput")

    T = sched["T"]
    blk_meta = sched["blk_meta"]
    t_c = cfg.t_c

    with tile.TileContext(nc) as tc:
        with (
            tc.tile_pool(name="dram", bufs=1, space="DRAM") as dpool,
            tc.tile_pool(name="const", bufs=1) as cpool,
            tc.tile_pool(name="spsum", bufs=4, space="PSUM") as spp,
        ):
            H = [dpool.tile([P, t_c, OUT_DIM], f16, tag=f"h{c}", name=f"hbuf{c}")
                 for c in range(NCH)]
            H_flat = [h[:].rearrange("p t f -> (p t) f") for h in H]

            wt0 = cpool.tile([P, OUT_DIM], f16)
            wt1 = cpool.tile([P, OUT_DIM], f16)
            nc.sync.dma_start(out=wt0[:], in_=WT[0:P, :])
            nc.sync.dma_start(out=wt1[:], in_=WT[P:2 * P, :])
            brow_t = cpool.tile([1, OUT_DIM], f16)
            nc.sync.dma_start(out=brow_t[:], in_=BR[:, :])
            iota_t = cpool.tile([P, P], f16)
            nc.sync.dma_start(out=iota_t[:], in_=IO[:, :])

            # ---------------- phase 1: H' = X @ W^T, chunk by chunk
            QUAD = 4
            SLAB = 98             # row tiles per XT slab DMA (t_c = 196 = 2*98)
            with (
                tc.tile_pool(name="xt", bufs=2) as xtp,
                tc.tile_pool(name="hpsum", bufs=2, space="PSUM") as hpp,
                tc.tile_pool(name="hstg", bufs=3) as hsp,
            ):
                for s0 in range(0, cfg.t_t, SLAB):
                    s1 = min(s0 + SLAB, cfg.t_t)
                    rows = (s1 - s0) * P
                    xt0 = xtp.tile([P, SLAB * P], f16, tag="x0")
                    xt1 = xtp.tile([P, SLAB * P], f16, tag="x1")
                    nc.sync.dma_start(out=xt0[:, :rows], in_=XT[0:P, s0 * P:s1 * P])
                    nc.sync.dma_start(out=xt1[:, :rows], in_=XT[P:2 * P, s0 * P:s1 * P])
                    q0 = s0
                    while q0 < s1:
                        cc = q0 // t_c
                        q1 = min(q0 + QUAD, s1, (cc + 1) * t_c)
                        nq = q1 - q0
                        ps = hpp.tile([P, QUAD * OUT_DIM], f32)
                        for qi, t in enumerate(range(q0, q1)):
                            o = (t - s0) * P
                            seg = ps[:, qi * OUT_DIM:(qi + 1) * OUT_DIM]
                            nc.tensor.matmul(seg, lhsT=xt0[:, o:o + P], rhs=wt0[:],
                                             start=True, stop=False)
                            nc.tensor.matmul(seg, lhsT=xt1[:, o:o + P], rhs=wt1[:],
                                             start=False, stop=True)
                        hst = hsp.tile([P, QUAD, OUT_DIM], f16)
                        nc.any.tensor_copy(
                            hst[:, :nq, :],
                            ps[:, :nq * OUT_DIM].rearrange("p (q f) -> p q f", f=OUT_DIM))
                        u0 = q0 % t_c
                        assert (q1 - 1) // t_c == cc
                        nc.sync.dma_start(out=H[cc][:, u0:u0 + nq, :],
                                          in_=hst[:, :nq, :])
                        q0 = q1

            # ---------------- phase 2: gather + selector-matmul segment sum
            with (
                tc.tile_pool(name="sec", bufs=6) as secp,
                tc.tile_pool(name="side", bufs=2) as sidep,
                tc.tile_pool(name="st", bufs=10) as stp,
                tc.tile_pool(name="ostg", bufs=3) as opool,
            ):
                gq = 0
                for bi, bm in enumerate(blk_meta):
                    nsl = bm["n_slots"]
                    blk = bm["blk"]
                    nbg = len(blk)
                    blk_slot0 = bm["blk_slot0"]
                    g0 = blk[0]
                    q_lo = int(sched["g_qoff"][g0])
                    q_hi = q_lo + int(sum(sched["g_tiles"][gi] for gi in blk))
                    # per-block sideband loads
                    lrg_t = sidep.tile([P, sched["tg_max"] * cfg.block_groups],
                                       f16, tag="lrg")
                    val_t = sidep.tile([P, sched["sec_max"] * NCH], f16, tag="val")
                    cols_t = sidep.tile([P, sched["sec_max"] * NCH * 8], dt.int16,
                                        tag="cols")
                    deg_t = sidep.tile([1, cfg.block_groups * P], f16, tag="deg")
                    if q_hi > q_lo:
                        nc.sync.dma_start(out=lrg_t[:, :q_hi - q_lo],
                                          in_=LG[:, q_lo:q_hi])
                    if nsl:
                        nc.sync.dma_start(out=val_t[:, :nsl],
                                          in_=VA[:, blk_slot0:blk_slot0 + nsl])
                        nc.sync.dma_start(
                            out=cols_t[:, :nsl * 8],
                            in_=CO[:, blk_slot0 * 8:(blk_slot0 + nsl) * 8])
                    nc.sync.dma_start(out=deg_t[:, :nbg * P],
                                      in_=DG[:, g0 * P:g0 * P + nbg * P])

                    # S_T per group (only needs lrg/iota — can run early)
                    st_tiles = {}
                    for gi in blk:
                        tg = int(sched["g_tiles"][gi])
                        st = stp.tile([P, sched["tg_max"] * P], f16, tag="st")
                        st_tiles[gi] = (st, tg)
                        if tg == 0:
                            continue
                        ql = int(sched["g_qoff"][gi]) - q_lo
                        nc.vector.tensor_tensor(
                            out=st[:, :tg * P].rearrange("p (t r) -> p t r", r=P),
                            in0=lrg_t[:, ql:ql + tg, None].to_broadcast([P, tg, P]),
                            in1=iota_t[:, None, :].to_broadcast([P, tg, P]),
                            op=mybir.AluOpType.is_equal,
                        )

                    # per-chunk sections: gather -> val multiply
                    mt_tiles = {}
                    for sec in bm["secs"]:
                        cc, sns = sec["c"], sec["n_slots"]
                        if sns == 0:
                            continue
                        soff = sec["slot0"] - blk_slot0
                        mt = secp.tile([P, sched["sec_max"], OUT_DIM], f16, tag="m")
                        mt_tiles[cc] = mt
                        for a in range(0, sns, cfg.max_gather_slots):
                            k = min(cfg.max_gather_slots, sns - a)
                            nc.gpsimd.dma_gather(
                                out_ap=mt[:, a:a + k, :],
                                in_ap=H_flat[cc],
                                idxs_ap=cols_t[:, (soff + a) * 8:(soff + a + k) * 8],
                                num_idxs=k * P,
                                num_idxs_reg=k * P,
                                elem_size=OUT_DIM,
                                queue_num=gq % cfg.n_queues,
                            )
                            gq += 1
                        nc.any.tensor_tensor(
                            out=mt[:, :sns, :],
                            in0=mt[:, :sns, :],
                            in1=val_t[:, soff:soff + sns, None]
                            .to_broadcast([P, sns, OUT_DIM]),
                            op=mybir.AluOpType.mult,
                        )

                    # matmuls: per group, one contiguous accumulation chain
                    psums = {}
                    for pi in range(0, nbg, 4):
                        ps = spp.tile([P, 4 * OUT_DIM], f32)
                        for gi in blk[pi:pi + 4]:
                            j = blk.index(gi)
                            psums[gi] = ps[:, (j % 4) * OUT_DIM:(j % 4 + 1) * OUT_DIM]
                            tg = int(sched["g_tiles"][gi])
                            nc.tensor.matmul(
                                psums[gi],
                                lhsT=deg_t[0:1, j * P:(j + 1) * P],
                                rhs=brow_t[0:1, :],
                                start=True, stop=tg == 0)
                            st, _tg = st_tiles[gi]
                            kk = 0
                            for cc in range(NCH):
                                o_gc = int(T[[x for x in blk if x < gi], cc].sum())
                                n_t = int(T[gi, cc])
                                for k in range(n_t):
                                    nc.tensor.matmul(
                                        psums[gi],
                                        lhsT=st[:, kk * P:(kk + 1) * P],
                                        rhs=mt_tiles[cc][:, o_gc + k, :],
                                        start=False,
                                        stop=kk == tg - 1,
                                    )
                                    kk += 1

                    # evict: psum -> sbuf f32 -> OUT
                    for pi in range(0, nbg, 4):
                        npg = min(4, nbg - pi)
                        og = opool.tile([P, 4, OUT_DIM], f32)
                        for j in range(npg):
                            nc.any.tensor_copy(og[:, j, :], psums[blk[pi + j]])
                        nc.sync.dma_start(
                            out=OUT[blk[pi] * P:(blk[pi] + npg) * P, :]
                            .rearrange("(q p) f -> p q f", p=P),
                            in_=og[:, :npg, :])

    nc.finalize()
    return nc


# ---------------------------------------------------------------- driver
_CACHE = {}


def _ensure_ntff_hook():
    """Provide antenv.axon_hooks + the ctypes NTFF profile hook when the
    agent image lacks them (needed only for trace=True)."""
    import sys
    import types
    import contextlib
    import ctypes
    try:
        from antenv.axon_hooks import get_axon_ntff_profile_hook  # noqa: F401
        return
    except ImportError:
        pass
    import antenv
    mod = types.ModuleType("antenv.axon_hooks")
    mod._hook = None

    def set_axon_ntff_profile_hook(h):
        mod._hook = h

    def get_axon_ntff_profile_hook():
        return mod._hook

    mod.set_axon_ntff_profile_hook = set_axon_ntff_profile_hook
    mod.get_axon_ntff_profile_hook = get_axon_ntff_profile_hook
    sys.modules["antenv.axon_hooks"] = mod
    antenv.axon_hooks = mod

    so_path = "/opt/axon/libaxon_pjrt.so"
    try:
        lib = ctypes.CDLL(so_path)
    except OSError:
        return
    if not hasattr(lib, "axon_start_nrt_profile"):
        return
    lib.axon_start_nrt_profile.argtypes = [ctypes.POINTER(ctypes.c_int64),
                                           ctypes.c_size_t]
    lib.axon_start_nrt_profile.restype = ctypes.c_int64
    lib.axon_stop_nrt_profile.argtypes = [ctypes.c_char_p]
    lib.axon_stop_nrt_profile.restype = ctypes.c_int64

    @contextlib.contextmanager
    def _hook(output_dir, device_ids):
        import jax
        jax.devices()
        if device_ids:
            ids = (ctypes.c_int64 * len(device_ids))(*device_ids)
            rc = lib.axon_start_nrt_profile(ids, len(device_ids))
        else:
            rc = lib.axon_start_nrt_profile(None, 0)
        if rc != 0:
            raise RuntimeError(f"axon_start_nrt_profile rc={rc}")
        try:
            yield
        finally:
            n = lib.axon_stop_nrt_profile(str(output_dir).encode())
            print(f"ntff profile: {n} file(s) written to {output_dir}", flush=True)

    set_axon_ntff_profile_hook(_hook)


def _prep_inputs(cfg, X, W, b, sched, data):
    Xf = np.asarray(X, np.float32).astype(np.float16)
    n = Xf.shape[0]
    xt = np.zeros((IN_DIM, cfg.n_nodes_pad), dtype=np.float16)
    narr = np.arange(n)
    cch = narr % NCH
    qq = narr // NCH
    col = (cch * cfg.t_c + qq // P) * P + (qq % P)
    xt[:, col] = Xf.T
    wt = np.asarray(W, np.float32).T.astype(np.float16)
    brow = np.asarray(b, np.float32).astype(np.float16)[None, :]
    iota = np.tile(np.arange(P, dtype=np.float16)[None, :], (P, 1))
    in_maps = []
    for ci in range(N_CORES):
        in_maps.append({
            "xt": xt, "wt": wt, "brow": brow, "iota": iota,
            "cols": data["cols"][ci], "lrg": data["lrg"][ci],
            "val": data["val"][ci], "deg": data["deg"][ci],
        })
    return in_maps


def run(X, edge_row, edge_col, edge_val, W, b, cfg=DEFAULT_CFG, trace=False):
    if trace:
        _ensure_ntff_hook()
    sched, data = build_schedule(cfg, edge_row, edge_col, edge_val)
    key = ("prog", cfg.n_nodes, cfg.rows_per_core, sched["tot_slots"],
           tuple(sched["T"].reshape(-1)))
    if key not in _CACHE:
        _CACHE.clear()
        _CACHE[key] = build_program(cfg, sched)
    nc = _CACHE[key]
    in_maps = _prep_inputs(cfg, X, W, b, sched, data)
    res = run_bass_kernel_spmd(nc, in_maps, core_ids=list(range(N_CORES)),
                               trace=trace)
    outs = [res.results[ci]["out"][:cfg.rows_per_core] for ci in range(N_CORES)]
    full = np.concatenate(outs, axis=0).astype(np.float32)
    return full, res


def kernel(X, edge_row, edge_col, edge_val, W, b):
    out, _ = run(X, edge_row, edge_col, edge_val, W, b)
    return out
